# revision 8
# baseline (speedup 1.0000x reference)
"""Bahdanau additive attention on 8 TRN2 NeuronCores, data-parallel over batch.

reference:
    h1 = enc @ W1 + b1              [B,S,U]
    h2 = hid @ W2 + b2              [B,1,U]
    score = tanh(h1+h2) @ V + bv    [B,S,1]   (bv dropped: softmax-invariant)
    w = softmax(score, axis=S)
    ctx = sum_s w * enc             [B,D]

Per-core (4 batches): enc tiles cast-DMA'd f32->bf16 and kept in SBUF.
PE transposes (via identity) build encT [d,s]; h1T accumulated over d-chunks
in PSUM (bf16, N=512); ScalarE tanh with per-partition bias h2T+b1+b2
(h2 computed on-device from W2/hid in f32); scores via V.T @ tanh on PE;
softmax without max-subtraction (scores are O(1)); ctx = esc.T @ enc_native
from the cached bf16 tiles, scaled by 1/sum(exp).
"""
import sys
import numpy as np
from contextlib import ExitStack

if "/opt/trn_rl_repo" not in sys.path:
    sys.path.insert(0, "/opt/trn_rl_repo")

import ml_dtypes
from concourse import bacc, mybir, tile
from concourse.bass_utils import run_bass_kernel_spmd
from concourse.masks import make_identity

F32 = mybir.dt.float32
BF16 = mybir.dt.bfloat16
BF16NP = ml_dtypes.bfloat16

B, S, D, U = 32, 2048, 1024, 1024
NCORES = 8
BL = B // NCORES          # 4 batches per core
P = 128
KD = D // P               # 8 d-chunks
KU = U // P               # 8 u-chunks
NT = 512                  # matmul free-dim tile
ST = S // NT              # 4 s-tiles per batch
SB = S // P               # 16 s-blocks of 128

_NC_CACHE = None
LAST_RESULT = None        # test.py reads exec_time_ns off this
TRACE_DIR = None          # when set (and BASS_TRACE=1), ntff profile lands here


def _build():
    nc = bacc.Bacc("TRN2", target_bir_lowering=False)

    enc_in = nc.dram_tensor("enc", [BL, S, D], F32, kind="ExternalInput")
    w1_in = nc.dram_tensor("w1r", [P, KD * U], BF16, kind="ExternalInput")
    w2_in = nc.dram_tensor("w2r", [P, KD * U], BF16, kind="ExternalInput")
    hidT_in = nc.dram_tensor("hidT", [P, KD * BL], BF16, kind="ExternalInput")
    b1T_in = nc.dram_tensor("b1T", [P, KU], F32, kind="ExternalInput")
    b2T_in = nc.dram_tensor("b2T", [P, KU], F32, kind="ExternalInput")
    vT_in = nc.dram_tensor("vT", [P, KU], BF16, kind="ExternalInput")
    out_ext = nc.dram_tensor("out", [BL, D], F32, kind="ExternalOutput")

    with tile.TileContext(nc) as tc, ExitStack() as ctx:
        const = ctx.enter_context(tc.tile_pool(name="const", bufs=1))
        nat_pool = ctx.enter_context(tc.tile_pool(name="nat", bufs=20))
        encT_pool = ctx.enter_context(tc.tile_pool(name="encT", bufs=2))
        tanh_pool = ctx.enter_context(tc.tile_pool(name="tanh", bufs=3))
        sc_pool = ctx.enter_context(tc.tile_pool(name="sc", bufs=2))
        small = ctx.enter_context(tc.tile_pool(name="small", bufs=4))
        out_pool = ctx.enter_context(tc.tile_pool(name="outp", bufs=2))

        ps_tr = ctx.enter_context(tc.tile_pool(name="ps_tr", bufs=2, space="PSUM"))
        ps_h1 = ctx.enter_context(tc.tile_pool(name="ps_h1", bufs=2, space="PSUM"))
        ps_sc = ctx.enter_context(tc.tile_pool(name="ps_sc", bufs=1, space="PSUM"))
        ps_misc = ctx.enter_context(tc.tile_pool(name="ps_misc", bufs=1, space="PSUM"))
        ps_ctx = ctx.enter_context(tc.tile_pool(name="ps_ctx", bufs=1, space="PSUM"))

        # ---- constants ----
        ident = const.tile([P, P], BF16)
        make_identity(nc, ident[:])
        one1 = const.tile([1, 1], F32)
        nc.any.memset(one1[:], 1.0)
        ones128 = const.tile([P, 1], BF16)
        nc.any.memset(ones128[:], 1.0)
        w2_sb = const.tile([P, KD * U], BF16)
        nc.sync.dma_start(w2_sb[:], w2_in[:])
        hidT_sb = const.tile([P, KD * BL], BF16)
        nc.sync.dma_start(hidT_sb[:], hidT_in[:])
        b1T_sb = const.tile([P, KU], F32)
        nc.sync.dma_start(b1T_sb[:], b1T_in[:])
        b2T_sb = const.tile([P, KU], F32)
        nc.sync.dma_start(b2T_sb[:], b2T_in[:])
        vT_sb = const.tile([P, KU], BF16)
        nc.sync.dma_start(vT_sb[:], vT_in[:])
        w1_sb = const.tile([P, KD * U], BF16)
        nc.sync.dma_start(w1_sb[:], w1_in[:])
        bias_sb = const.tile([P, KU * BL], F32)   # bias[u(m,p), m*BL+b]

        def emit_h2():
            # h2T = W2.T @ hidT + b1 + b2 (bf16 matmuls, f32 psum/bias)
            for m in range(KU):
                ph2 = ps_misc.tile([P, BL], F32, tag="misc")
                for k in range(KD):
                    nc.tensor.matmul(
                        ph2[:], w2_sb[:, k * U + m * P:k * U + (m + 1) * P],
                        hidT_sb[:, k * BL:(k + 1) * BL],
                        start=(k == 0), stop=(k == KD - 1))
                nc.vector.tensor_scalar(
                    bias_sb[:, m * BL:(m + 1) * BL], ph2[:],
                    b1T_sb[:, m:m + 1], b2T_sb[:, m:m + 1],
                    mybir.AluOpType.add, mybir.AluOpType.add)

        # ---- main per-batch pipeline ----
        def emit_transposes(nat_tiles, t, encT):
            """encT[:, k, j*128:(j+1)*128] = nat[t*4+j][:, k*128:(k+1)*128].T

            Done as regular matmuls against the identity (out = natchunk.T @ I):
            keeps the PE HAM activity monitor warm and pipelines at ~81ns/op,
            unlike transpose-mode (~236ns, doesn't count as PE-busy).
            """
            for k in range(KD):
                pt = ps_tr.tile([P, NT], F32)
                for j in range(NT // P):
                    nc.tensor.matmul(
                        pt[:, j * P:(j + 1) * P],
                        nat_tiles[t * (NT // P) + j][:, k * P:(k + 1) * P],
                        ident[:], start=True, stop=True)
                nc.vector.tensor_copy(encT[:, k, :], pt[:])

        for b in range(BL):
            nat_tiles = []
            for st in range(SB):
                nt_t = nat_pool.tile([P, D], BF16)
                nc.gpsimd.dma_start(nt_t[:], enc_in[b, st * P:(st + 1) * P, :])
                nat_tiles.append(nt_t)

            score_sb = sc_pool.tile([1, S], F32)
            encT = encT_pool.tile([P, KD, NT], BF16)
            emit_transposes(nat_tiles, 0, encT)
            if b == 0:
                emit_h2()
            for t in range(ST):
                psum_sc = ps_sc.tile([1, NT], F32)
                tanh_tiles = {}
                for m in range(KU):
                    ph1 = ps_h1.tile([P, NT], F32)
                    for k in range(KD):
                        nc.tensor.matmul(
                            ph1[:], w1_sb[:, k * U + m * P:k * U + (m + 1) * P],
                            encT[:, k, :],
                            start=(k == 0), stop=(k == KD - 1))
                    tanh_t = tanh_pool.tile([P, NT], BF16)
                    nc.scalar.activation(
                        tanh_t[:], ph1[:], mybir.ActivationFunctionType.Tanh,
                        bias=bias_sb[:, m * BL + b:m * BL + b + 1], scale=1.0)
                    tanh_tiles[m] = tanh_t
                    if m == 0 and t < ST - 1:
                        encT_next = encT_pool.tile([P, KD, NT], BF16)
                        emit_transposes(nat_tiles, t + 1, encT_next)
                    if m >= 1:
                        nc.tensor.matmul(
                            psum_sc[:], vT_sb[:, m - 1:m], tanh_tiles[m - 1][:],
                            start=(m - 1 == 0), stop=False)
                nc.tensor.matmul(
                    psum_sc[:], vT_sb[:, KU - 1:KU], tanh_tiles[KU - 1][:],
                    start=False, stop=True)
                nc.vector.tensor_copy(score_sb[:1, t * NT:(t + 1) * NT], psum_sc[:])
                if t < ST - 1:
                    encT = encT_next

            # scoreT: [1, S] -> [128, 16]
            psum_sT = ps_misc.tile([P, SB], F32, tag="misc")
            for jj in range(SB):
                nc.tensor.matmul(
                    psum_sT[:, jj:jj + 1], score_sb[:1, jj * P:(jj + 1) * P],
                    one1[:1, :1], start=True, stop=True)
            esc = small.tile([P, SB], BF16)
            rowsum = small.tile([P, 1], F32)
            nc.scalar.activation(
                esc[:], psum_sT[:], mybir.ActivationFunctionType.Exp,
                accum_out=rowsum[:])
            rs_bf = small.tile([P, 1], BF16)
            nc.vector.tensor_copy(rs_bf[:], rowsum[:])
            psum_s1 = ps_misc.tile([1, 1], F32, tag="misc")
            nc.tensor.matmul(psum_s1[:], rs_bf[:, :], ones128[:, :1],
                             start=True, stop=True)
            sum_sb = small.tile([1, 1], F32)
            nc.vector.tensor_copy(sum_sb[:], psum_s1[:])
            rinv = small.tile([1, 1], F32)
            nc.vector.reciprocal(rinv[:], sum_sb[:])

            # ctx = esc.T @ enc (native tiles), scaled by 1/sum
            pc = [ps_ctx.tile([1, NT], F32, name=f"pc{h}", tag=f"pc{h}")
                  for h in range(D // NT)]
            for j in range(SB):
                for h in range(D // NT):
                    nc.tensor.matmul(
                        pc[h][:], esc[:, j:j + 1],
                        nat_tiles[j][:, h * NT:(h + 1) * NT],
                        start=(j == 0), stop=(j == SB - 1))
            out_t = out_pool.tile([1, D], F32)
            for h in range(D // NT):
                nc.vector.tensor_scalar_mul(
                    out_t[:1, h * NT:(h + 1) * NT], pc[h][:], rinv[:1, :1])
            nc.sync.dma_start(out_ext[b:b + 1, :], out_t[:1, :])

    nc.compile()
    return nc


def _get_nc():
    global _NC_CACHE
    if _NC_CACHE is None:
        _NC_CACHE = _build()
    return _NC_CACHE


def kernel(**inputs):
    global LAST_RESULT
    enc = np.asarray(inputs["enc"], dtype=np.float32)
    hid = np.asarray(inputs["hid"], dtype=np.float32)
    W1 = np.asarray(inputs["W1"], dtype=np.float32)
    b1 = np.asarray(inputs["b1"], dtype=np.float32)
    W2 = np.asarray(inputs["W2"], dtype=np.float32)
    b2 = np.asarray(inputs["b2"], dtype=np.float32)
    V = np.asarray(inputs["V"], dtype=np.float32)
    # bv shifts all scores of a batch equally -> softmax unchanged; unused.

    # host-side layout prep (pure reshapes/transposes of tiny tensors)
    w1r = np.ascontiguousarray(
        W1.reshape(KD, P, U).transpose(1, 0, 2).reshape(P, KD * U)
    ).astype(BF16NP)
    w2r = np.ascontiguousarray(
        W2.reshape(KD, P, U).transpose(1, 0, 2).reshape(P, KD * U)).astype(BF16NP)
    b1T = np.ascontiguousarray(b1.reshape(KU, P).T)
    b2T = np.ascontiguousarray(b2.reshape(KU, P).T)
    vT = np.ascontiguousarray(V.reshape(KU, P).T).astype(BF16NP)

    nc = _get_nc()
    in_maps = []
    for i in range(NCORES):
        hs = hid[i * BL:(i + 1) * BL]                       # [BL, D]
        hidT = np.ascontiguousarray(
            hs.reshape(BL, KD, P).transpose(2, 1, 0).reshape(P, KD * BL)).astype(BF16NP)
        in_maps.append({
            "enc": np.ascontiguousarray(enc[i * BL:(i + 1) * BL]),
            "w1r": w1r, "w2r": w2r, "hidT": hidT,
            "b1T": b1T, "b2T": b2T, "vT": vT,
        })
    kwargs = {}
    if TRACE_DIR is not None:
        kwargs["tmpdir"] = TRACE_DIR
    res = run_bass_kernel_spmd(nc, in_maps, list(range(NCORES)), **kwargs)
    LAST_RESULT = res
    out = np.concatenate([res.results[i]["out"] for i in range(NCORES)], axis=0)
    return out.astype(np.float32)


# revision 10
# speedup vs baseline: 1.1222x; 1.1222x over previous
"""Bahdanau additive attention on 8 TRN2 NeuronCores, data-parallel over batch.

reference:
    h1 = enc @ W1 + b1              [B,S,U]
    h2 = hid @ W2 + b2              [B,1,U]
    score = tanh(h1+h2) @ V + bv    [B,S,1]   (bv dropped: softmax-invariant)
    w = softmax(score, axis=S)
    ctx = sum_s w * enc             [B,D]

Per-core (4 batches): enc tiles cast-DMA'd f32->bf16 and kept in SBUF.
PE transposes (via identity) build encT [d,s]; h1T accumulated over d-chunks
in PSUM (bf16, N=512); ScalarE tanh with per-partition bias h2T+b1+b2
(h2 computed on-device from W2/hid in f32); scores via V.T @ tanh on PE;
softmax without max-subtraction (scores are O(1)); ctx = esc.T @ enc_native
from the cached bf16 tiles, scaled by 1/sum(exp).
"""
import sys
import numpy as np
from contextlib import ExitStack

if "/opt/trn_rl_repo" not in sys.path:
    sys.path.insert(0, "/opt/trn_rl_repo")

import ml_dtypes
from concourse import bacc, mybir, tile
from concourse.bass_utils import run_bass_kernel_spmd
from concourse.masks import make_identity

F32 = mybir.dt.float32
BF16 = mybir.dt.bfloat16
BF16NP = ml_dtypes.bfloat16

B, S, D, U = 32, 2048, 1024, 1024
NCORES = 8
BL = B // NCORES          # 4 batches per core
P = 128
KD = D // P               # 8 d-chunks
KU = U // P               # 8 u-chunks
NT = 512                  # matmul free-dim tile
ST = S // NT              # 4 s-tiles per batch
SB = S // P               # 16 s-blocks of 128

_NC_CACHE = None
LAST_RESULT = None        # test.py reads exec_time_ns off this
TRACE_DIR = None          # when set (and BASS_TRACE=1), ntff profile lands here


def _build():
    nc = bacc.Bacc("TRN2", target_bir_lowering=False)

    enc_in = nc.dram_tensor("enc", [BL, S, D], F32, kind="ExternalInput")
    w1_in = nc.dram_tensor("w1r", [P, KD * U], BF16, kind="ExternalInput")
    w2_in = nc.dram_tensor("w2r", [P, KD * U], BF16, kind="ExternalInput")
    hidT_in = nc.dram_tensor("hidT", [P, KD * BL], BF16, kind="ExternalInput")
    b1T_in = nc.dram_tensor("b1T", [P, KU], F32, kind="ExternalInput")
    b2T_in = nc.dram_tensor("b2T", [P, KU], F32, kind="ExternalInput")
    vT_in = nc.dram_tensor("vT", [P, KU], BF16, kind="ExternalInput")
    out_ext = nc.dram_tensor("out", [BL, D], F32, kind="ExternalOutput")

    with tile.TileContext(nc) as tc, ExitStack() as ctx:
        const = ctx.enter_context(tc.tile_pool(name="const", bufs=1))
        nat_pool = ctx.enter_context(tc.tile_pool(name="nat", bufs=20))
        encT_pool = ctx.enter_context(tc.tile_pool(name="encT", bufs=2))
        tanh_pool = ctx.enter_context(tc.tile_pool(name="tanh", bufs=3))
        sc_pool = ctx.enter_context(tc.tile_pool(name="sc", bufs=2))
        vacc_pool = ctx.enter_context(tc.tile_pool(name="vacc", bufs=2))
        small = ctx.enter_context(tc.tile_pool(name="small", bufs=4))
        out_pool = ctx.enter_context(tc.tile_pool(name="outp", bufs=2))

        ps_tr = ctx.enter_context(tc.tile_pool(name="ps_tr", bufs=2, space="PSUM"))
        ps_h1 = ctx.enter_context(tc.tile_pool(name="ps_h1", bufs=2, space="PSUM"))
        ps_sc = ctx.enter_context(tc.tile_pool(name="ps_sc", bufs=1, space="PSUM"))
        ps_misc = ctx.enter_context(tc.tile_pool(name="ps_misc", bufs=1, space="PSUM"))
        ps_ctx = ctx.enter_context(tc.tile_pool(name="ps_ctx", bufs=1, space="PSUM"))

        # ---- constants ----
        ident = const.tile([P, P], BF16)
        make_identity(nc, ident[:])
        one1 = const.tile([1, 1], F32)
        nc.any.memset(one1[:], 1.0)
        ones128 = const.tile([P, 1], BF16)
        nc.any.memset(ones128[:], 1.0)
        w1_sb = const.tile([P, KD * U], BF16)
        nc.sync.dma_start(w1_sb[:], w1_in[:])
        w2_sb = const.tile([P, KD * U], BF16)
        nc.scalar.dma_start(w2_sb[:], w2_in[:])
        hidT_sb = const.tile([P, KD * BL], BF16)
        nc.scalar.dma_start(hidT_sb[:], hidT_in[:])
        b1T_sb = const.tile([P, KU], F32)
        nc.scalar.dma_start(b1T_sb[:], b1T_in[:])
        b2T_sb = const.tile([P, KU], F32)
        nc.scalar.dma_start(b2T_sb[:], b2T_in[:])
        vT_sb = const.tile([P, KU], BF16)
        nc.scalar.dma_start(vT_sb[:], vT_in[:])
        v32_sb = const.tile([P, KU], F32)
        nc.vector.tensor_copy(v32_sb[:], vT_sb[:])
        bias_sb = const.tile([P, KU * BL], F32)   # bias[u(m,p), m*BL+b]

        def emit_h2():
            # h2T = W2.T @ hidT + b1 + b2 (bf16 matmuls, f32 psum/bias)
            for m in range(KU):
                ph2 = ps_misc.tile([P, BL], F32, tag="misc")
                for k in range(KD):
                    nc.tensor.matmul(
                        ph2[:], w2_sb[:, k * U + m * P:k * U + (m + 1) * P],
                        hidT_sb[:, k * BL:(k + 1) * BL],
                        start=(k == 0), stop=(k == KD - 1))
                nc.vector.tensor_scalar(
                    bias_sb[:, m * BL:(m + 1) * BL], ph2[:],
                    b1T_sb[:, m:m + 1], b2T_sb[:, m:m + 1],
                    mybir.AluOpType.add, mybir.AluOpType.add)

        # ---- main per-batch pipeline ----
        def emit_transposes(nat_tiles, t, encT):
            """encT[:, k, j*128:(j+1)*128] = nat[t*4+j][:, k*128:(k+1)*128].T

            Done as regular matmuls against the identity (out = natchunk.T @ I):
            keeps the PE HAM activity monitor warm and pipelines at ~81ns/op,
            unlike transpose-mode (~236ns, doesn't count as PE-busy).
            """
            for k in range(KD):
                pt = ps_tr.tile([P, NT], F32)
                for j in range(NT // P):
                    nc.tensor.matmul(
                        pt[:, j * P:(j + 1) * P],
                        nat_tiles[t * (NT // P) + j][:, k * P:(k + 1) * P],
                        ident[:], start=True, stop=True)
                nc.vector.tensor_copy(encT[:, k, :], pt[:])

        for b in range(BL):
            nat_tiles = []
            for st in range(SB):
                nt_t = nat_pool.tile([P, D], BF16)
                nc.gpsimd.dma_start(nt_t[:], enc_in[b, st * P:(st + 1) * P, :])
                nat_tiles.append(nt_t)

            score_sb = sc_pool.tile([1, S], F32)
            encT = encT_pool.tile([P, KD, NT], BF16)
            emit_transposes(nat_tiles, 0, encT)
            if b == 0:
                emit_h2()
            for t in range(ST):
                psum_sc = ps_sc.tile([1, NT], F32)
                vacc = vacc_pool.tile([P, NT], BF16)
                for m in range(KU):
                    ph1 = ps_h1.tile([P, NT], F32)
                    for k in range(KD):
                        nc.tensor.matmul(
                            ph1[:], w1_sb[:, k * U + m * P:k * U + (m + 1) * P],
                            encT[:, k, :],
                            start=(k == 0), stop=(k == KD - 1))
                    tanh_t = tanh_pool.tile([P, NT], BF16)
                    nc.scalar.activation(
                        tanh_t[:], ph1[:], mybir.ActivationFunctionType.Tanh,
                        bias=bias_sb[:, m * BL + b:m * BL + b + 1], scale=1.0)
                    if m == 0:
                        nc.vector.tensor_scalar_mul(
                            vacc[:], tanh_t[:], v32_sb[:, 0:1])
                    else:
                        nc.vector.scalar_tensor_tensor(
                            vacc[:], tanh_t[:], v32_sb[:, m:m + 1], vacc[:],
                            mybir.AluOpType.mult, mybir.AluOpType.add)
                    if m == 0 and t < ST - 1:
                        encT_next = encT_pool.tile([P, KD, NT], BF16)
                        emit_transposes(nat_tiles, t + 1, encT_next)
                nc.tensor.matmul(psum_sc[:], ones128[:, :1], vacc[:],
                                 start=True, stop=True)
                nc.vector.tensor_copy(score_sb[:1, t * NT:(t + 1) * NT], psum_sc[:])
                if t < ST - 1:
                    encT = encT_next

            # scoreT: [1, S] -> [128, 16]
            psum_sT = ps_misc.tile([P, SB], F32, tag="misc")
            for jj in range(SB):
                nc.tensor.matmul(
                    psum_sT[:, jj:jj + 1], score_sb[:1, jj * P:(jj + 1) * P],
                    one1[:1, :1], start=True, stop=True)
            esc = small.tile([P, SB], BF16)
            rowsum = small.tile([P, 1], F32)
            nc.scalar.activation(
                esc[:], psum_sT[:], mybir.ActivationFunctionType.Exp,
                accum_out=rowsum[:])
            rs_bf = small.tile([P, 1], BF16)
            nc.vector.tensor_copy(rs_bf[:], rowsum[:])
            psum_s1 = ps_misc.tile([1, 1], F32, tag="misc")
            nc.tensor.matmul(psum_s1[:], rs_bf[:, :], ones128[:, :1],
                             start=True, stop=True)
            sum_sb = small.tile([1, 1], F32)
            nc.vector.tensor_copy(sum_sb[:], psum_s1[:])
            rinv = small.tile([1, 1], F32)
            nc.vector.reciprocal(rinv[:], sum_sb[:])

            # ctx = esc.T @ enc (native tiles), scaled by 1/sum
            pc = [ps_ctx.tile([1, NT], F32, name=f"pc{h}", tag=f"pc{h}")
                  for h in range(D // NT)]
            for j in range(SB):
                for h in range(D // NT):
                    nc.tensor.matmul(
                        pc[h][:], esc[:, j:j + 1],
                        nat_tiles[j][:, h * NT:(h + 1) * NT],
                        start=(j == 0), stop=(j == SB - 1))
            out_t = out_pool.tile([1, D], F32)
            for h in range(D // NT):
                nc.vector.tensor_scalar_mul(
                    out_t[:1, h * NT:(h + 1) * NT], pc[h][:], rinv[:1, :1])
            nc.sync.dma_start(out_ext[b:b + 1, :], out_t[:1, :])

    nc.compile()
    return nc


def _get_nc():
    global _NC_CACHE
    if _NC_CACHE is None:
        _NC_CACHE = _build()
    return _NC_CACHE


def kernel(**inputs):
    global LAST_RESULT
    enc = np.asarray(inputs["enc"], dtype=np.float32)
    hid = np.asarray(inputs["hid"], dtype=np.float32)
    W1 = np.asarray(inputs["W1"], dtype=np.float32)
    b1 = np.asarray(inputs["b1"], dtype=np.float32)
    W2 = np.asarray(inputs["W2"], dtype=np.float32)
    b2 = np.asarray(inputs["b2"], dtype=np.float32)
    V = np.asarray(inputs["V"], dtype=np.float32)
    # bv shifts all scores of a batch equally -> softmax unchanged; unused.

    # host-side layout prep (pure reshapes/transposes of tiny tensors)
    w1r = np.ascontiguousarray(
        W1.reshape(KD, P, U).transpose(1, 0, 2).reshape(P, KD * U)
    ).astype(BF16NP)
    w2r = np.ascontiguousarray(
        W2.reshape(KD, P, U).transpose(1, 0, 2).reshape(P, KD * U)).astype(BF16NP)
    b1T = np.ascontiguousarray(b1.reshape(KU, P).T)
    b2T = np.ascontiguousarray(b2.reshape(KU, P).T)
    vT = np.ascontiguousarray(V.reshape(KU, P).T).astype(BF16NP)

    nc = _get_nc()
    in_maps = []
    for i in range(NCORES):
        hs = hid[i * BL:(i + 1) * BL]                       # [BL, D]
        hidT = np.ascontiguousarray(
            hs.reshape(BL, KD, P).transpose(2, 1, 0).reshape(P, KD * BL)).astype(BF16NP)
        in_maps.append({
            "enc": np.ascontiguousarray(enc[i * BL:(i + 1) * BL]),
            "w1r": w1r, "w2r": w2r, "hidT": hidT,
            "b1T": b1T, "b2T": b2T, "vT": vT,
        })
    kwargs = {}
    if TRACE_DIR is not None:
        kwargs["tmpdir"] = TRACE_DIR
    res = run_bass_kernel_spmd(nc, in_maps, list(range(NCORES)), **kwargs)
    LAST_RESULT = res
    out = np.concatenate([res.results[i]["out"] for i in range(NCORES)], axis=0)
    return out.astype(np.float32)
